# revision 7
# baseline (speedup 1.0000x reference)
"""Bahdanau-attention kernel for TRN2, data-parallel over 8 NeuronCores.

Math: the reference applies softmax over the LAST axis of scores, which has
size 1 — softmax over a singleton axis is identically 1.0 (exp(x-x)/exp(x-x)).
Therefore:
    attn_weights = ones(bs, sq, 21, 7, 1)
    attn_out     = attn_weights * keys = broadcast(keys, (bs, sq, 21, 7, 256))
independent of queries / masks / all projection weights. The kernel is a pure
DMA problem: per core, read its keys shard into SBUF and write it back 21x
(broadcast over the query axis), plus a ones fill for the weights output.

Per-core traffic: read ~2.5 MiB + write 36.9 MiB ~= 40.5 MB at the ~400 GB/s
achievable SDMA aggregate => ~103 us steady-state + ~8 us startup/tail.

Engine load balancing: SBUF-sourced descriptors are pinned to SDMA engines by
the partition port map (engine 0 <-> partitions {0-3,32-35}, engine 15 <->
{92-95,124-127}); on even-numbered cores one of those two engines runs ~20%
slow. DRAM->DRAM descriptors round-robin across all 16 engines. So partitions
of engines 0/15 emit only 18 of the 21 query copies from SBUF; their remaining
3 copies are issued as DRAM->DRAM broadcasts (spread over all engines, and
overlapped with the initial HBM->SBUF load).
"""

import numpy as np

from concourse import bass, mybir
from concourse.bass_utils import run_bass_kernel_spmd

BS, SQ, NQ, NK, D = 16, 128, 21, 7, 256
N_CORES = 8
BPC = BS // N_CORES  # batches per core
ROW = NK * D  # contiguous floats per (b, s): 1792
W_ROW = NQ * NK  # attn_weights floats per (b, s): 147

NQ_BASE = 18  # query copies emitted from SBUF by every partition
# partition blocks owned by SDMA engines 0 and 15 (slow on even cores):
SLOW_BLOCKS = (0, 32, 92, 124)
# contiguous partition ranges covering engines 1..14 only:
FAST_RANGES = ((4, 32), (36, 92), (96, 124))


def _build() -> bass.Bass:
    nc = bass.Bass()
    keys_in = nc.declare_dram_parameter(
        "keys", [BPC, SQ, ROW], mybir.dt.float32, isOutput=False
    )
    ones_in = nc.declare_dram_parameter(
        "ones", [SQ, W_ROW], mybir.dt.float32, isOutput=False
    )
    attn_out = nc.declare_dram_parameter(
        "attn_out", [BPC, SQ, NQ, ROW], mybir.dt.float32, isOutput=True
    )
    attn_w = nc.declare_dram_parameter(
        "attn_w", [BPC, SQ, W_ROW], mybir.dt.float32, isOutput=True
    )

    n_out = 0  # DMAs counted on out_sem

    with (
        nc.Block() as block,
        nc.semaphore("in_sem") as in_sem,
        nc.semaphore("out_sem") as out_sem,
        nc.sbuf_tensor("kt", [SQ, BPC, ROW], mybir.dt.float32) as kt,
    ):

        @block.sync
        def _(sync: bass.BassEngine):
            nonlocal n_out
            # keys shard -> SBUF, one DMA per batch so b=0 output can start
            # while b=1 still loads
            for b in range(BPC):
                sync.dma_start(out=kt[:, b], in_=keys_in[b]).then_inc(in_sem, 16)

            # --- DRAM->DRAM work first: overlaps the SBUF load ---
            # weights: ones broadcast over b, iterated (s, b, r)
            sync.dma_start(
                out=attn_w[:].transpose([1, 0, 2]),
                in_=ones_in[:].unsqueeze(1).broadcast_to((SQ, BPC, W_ROW)),
            ).then_inc(out_sem, 16)
            n_out += 1
            # q copies NQ_BASE..NQ-1 for the slow-engine partition blocks
            for blk in SLOW_BLOCKS:
                for b in range(BPC):
                    sync.dma_start(
                        out=attn_out[b, blk : blk + 4, NQ_BASE:NQ],
                        in_=keys_in[b, blk : blk + 4]
                        .unsqueeze(1)
                        .broadcast_to((4, NQ - NQ_BASE, ROW)),
                    ).then_inc(out_sem, 16)
                    n_out += 1

            # --- SBUF-sourced broadcast writes ---
            for b in range(BPC):
                sync.wait_ge(in_sem, 16 * (b + 1))
                # all 128 partitions: q copies 0..NQ_BASE-1
                sync.dma_start(
                    out=attn_out[b, :, :NQ_BASE],
                    in_=kt[:, b].unsqueeze(1).broadcast_to((SQ, NQ_BASE, ROW)),
                ).then_inc(out_sem, 16)
                n_out += 1
                # fast partition ranges: remaining q copies
                for lo, hi in FAST_RANGES:
                    sync.dma_start(
                        out=attn_out[b, lo:hi, NQ_BASE:NQ],
                        in_=kt[lo:hi, b]
                        .unsqueeze(1)
                        .broadcast_to((hi - lo, NQ - NQ_BASE, ROW)),
                    ).then_inc(out_sem, 16)
                    n_out += 1

            sync.wait_ge(out_sem, 16 * n_out)

    return nc


_NC_CACHE: list = []


def kernel(**inputs: np.ndarray):
    keys = np.ascontiguousarray(
        np.asarray(inputs["keys"], dtype=np.float32).reshape(BS, SQ, ROW)
    )
    if not _NC_CACHE:
        _NC_CACHE.append(_build())
    nc = _NC_CACHE[0]

    ones = np.ones((SQ, W_ROW), dtype=np.float32)
    in_maps = [
        {"keys": keys[c * BPC : (c + 1) * BPC], "ones": ones} for c in range(N_CORES)
    ]
    res = run_bass_kernel_spmd(nc, in_maps, core_ids=list(range(N_CORES)))

    attn_out = np.concatenate(
        [r["attn_out"].reshape(BPC, SQ, NQ, NK, D) for r in res.results], axis=0
    )
    attn_w = np.concatenate(
        [r["attn_w"].reshape(BPC, SQ, NQ, NK, 1) for r in res.results], axis=0
    )
    return attn_out, attn_w
